# revision 5
# baseline (speedup 1.0000x reference)
"""Trainium2 Bass kernel for nn_ConvAttention.

Module: key encoder (Conv 512->1024 k3 -> ReLU -> Conv 1024->80 k1) on text,
query encoder (Conv 80->160 k3 -> ReLU -> Conv 160->80 -> ReLU -> Conv 80->80)
on mels, L2-distance attention [B,Tm,Tt], log_softmax over Tt + log prior,
masked softmax.  Returns (attention, attention_logprob), both [8,1024,256] f32.

Sharding: data-parallel over batch B=8 -> one batch item per NeuronCore;
conv weights replicated (host-prepped into lhsT layouts, bf16).

Math notes (validated numerically against the jax reference):
  - sum_c (q-k)^2 = qq + kk - 2 qk; the qq term is constant along Tt so it
    cancels exactly in log_softmax -> never computed.
  - z = 0.001*qk - 0.0005*kk is computed by one PE matmul per Tm-tile with an
    augmented contraction row (q row of 1.0s against a row of -0.0005*kk);
    the 0.001 scale is folded into the third query conv's weights on host.
  - z in [-0.11, 0.0] for this input distribution -> exp() needs no
    max-subtraction (identical result up to fp rounding).
  - attention = softmax(where(mask, logsm + log(p+1e-8), -inf), Tt)
              = (e * (p+1e-8) * mask) / sum_tt(...)  with e = exp(z).
"""

import sys

sys.path.insert(0, "/opt/trn_rl_repo")

import numpy as np
import ml_dtypes

BF = ml_dtypes.bfloat16

B, CMEL, CTXT, TM, TT = 8, 80, 512, 1024, 256
N_CORES = 8

_STATE = {}


def _build():
    """Build + bacc-compile the single-core program (shared by all 8 cores)."""
    import concourse.bacc as bacc
    import concourse.tile as tile
    from concourse import mybir

    f32 = mybir.dt.float32
    bf16 = mybir.dt.bfloat16
    f32r = mybir.dt.float32r
    AF = mybir.ActivationFunctionType
    ALU = mybir.AluOpType

    nc = bacc.Bacc("TRN2", target_bir_lowering=False, debug=False,
                   num_devices=N_CORES)

    d_text = nc.dram_tensor("text_pad", [128, 4, 258], bf16, kind="ExternalInput").ap()
    d_mels = nc.dram_tensor("mels_pad", [80, 1026], bf16, kind="ExternalInput").ap()
    d_w1k = nc.dram_tensor("w1k", [128, 12, 1024], bf16, kind="ExternalInput").ap()
    d_w2k = nc.dram_tensor("w2k", [128, 8, 80], bf16, kind="ExternalInput").ap()
    d_wq1 = nc.dram_tensor("wq1", [80, 3, 160], bf16, kind="ExternalInput").ap()
    d_wq2 = nc.dram_tensor("wq2", [160, 80], bf16, kind="ExternalInput").ap()
    d_wq3 = nc.dram_tensor("wq3s", [80, 80], bf16, kind="ExternalInput").ap()
    d_kb1 = nc.dram_tensor("kb1c", [128, 8], f32, kind="ExternalInput").ap()
    d_kb2 = nc.dram_tensor("kb2c", [80, 1], f32, kind="ExternalInput").ap()
    d_qb1 = nc.dram_tensor("qb1c", [160, 1], f32, kind="ExternalInput").ap()
    d_qb2 = nc.dram_tensor("qb2c", [80, 1], f32, kind="ExternalInput").ap()
    d_qb3 = nc.dram_tensor("qb3c", [80, 1], f32, kind="ExternalInput").ap()
    d_mask = nc.dram_tensor("maskrep", [128, 256], f32, kind="ExternalInput").ap()
    d_prior = nc.dram_tensor("prior", [1024, 256], f32, kind="ExternalInput").ap()
    d_oatt = nc.dram_tensor("out_att", [1024, 256], f32, kind="ExternalOutput").ap()
    d_olp = nc.dram_tensor("out_lp", [1024, 256], f32, kind="ExternalOutput").ap()

    with tile.TileContext(nc) as tc:
        with (
            tc.tile_pool(name="w", bufs=1) as wp,
            tc.tile_pool(name="act", bufs=1) as acp,
            tc.tile_pool(name="sm", bufs=3) as sm,
            tc.tile_pool(name="ps", bufs=3, space="PSUM") as ps,
            tc.tile_pool(name="psq", bufs=2, space="PSUM") as psq,
            tc.tile_pool(name="psk", bufs=2, space="PSUM") as psk,
        ):
            # ---- input loads -------------------------------------------------
            mels_sb = wp.tile([80, 1026], bf16)
            nc.sync.dma_start(mels_sb[:], d_mels[:])
            wq1_sb = wp.tile([80, 3, 160], bf16)
            nc.sync.dma_start(wq1_sb[:], d_wq1[:])
            wq2a_sb = wp.tile([128, 80], bf16)
            nc.sync.dma_start(wq2a_sb[:], d_wq2[0:128, :])
            wq2b_sb = wp.tile([32, 80], bf16)
            nc.sync.dma_start(wq2b_sb[:], d_wq2[128:160, :])
            wq3_sb = wp.tile([80, 80], bf16)
            nc.sync.dma_start(wq3_sb[:], d_wq3[:])
            qb1a_sb = wp.tile([128, 1], f32)
            nc.sync.dma_start(qb1a_sb[:], d_qb1[0:128, :])
            qb1b_sb = wp.tile([32, 1], f32)
            nc.sync.dma_start(qb1b_sb[:], d_qb1[128:160, :])
            qb2_sb = wp.tile([80, 1], f32)
            nc.sync.dma_start(qb2_sb[:], d_qb2[:])
            qb3_sb = wp.tile([80, 1], f32)
            nc.sync.dma_start(qb3_sb[:], d_qb3[:])
            text_sb = wp.tile([128, 4, 258], bf16)
            nc.sync.dma_start(text_sb[:], d_text[:])
            w1k_sb = wp.tile([128, 12, 1024], bf16)
            for g in range(6):
                nc.sync.dma_start(w1k_sb[:, 2 * g:2 * g + 2, :],
                                  d_w1k[:, 2 * g:2 * g + 2, :])
            w2k_sb = wp.tile([128, 8, 80], bf16)
            nc.sync.dma_start(w2k_sb[:], d_w2k[:])
            kb1_sb = wp.tile([128, 8], f32)
            nc.sync.dma_start(kb1_sb[:], d_kb1[:])
            kb2_sb = wp.tile([80, 1], f32)
            nc.sync.dma_start(kb2_sb[:], d_kb2[:])
            mask_sb = wp.tile([128, 256], f32)
            nc.sync.dma_start(mask_sb[:], d_mask[:])

            ones80 = wp.tile([80, 1], f32)
            nc.vector.memset(ones80[:], 1.0)
            ones128 = wp.tile([1, 128], f32)
            nc.vector.memset(ones128[:], 1.0)
            eps_sb = wp.tile([128, 1], f32)
            nc.vector.memset(eps_sb[:], 1e-8)

            # ---- query encoder: mels [80,1024] -> qs_aug [81,1024] f32 -------
            # qs_aug rows 0..79 = 0.001 * q  (scale folded into wq3/qb3 on host),
            # row 80 = 1.0 (augmented contraction row for the kk term).
            yq1a = acp.tile([128, 1024], bf16)
            yq1b = acp.tile([32, 1024], bf16)
            yq2 = acp.tile([80, 1024], bf16)
            qs = acp.tile([80, 1024], f32)

            for nt in range(2):
                s = nt * 512
                qps = psq.tile([128, 512], f32, tag="qps")
                for dk in range(3):
                    nc.tensor.matmul(qps[:], wq1_sb[:, dk, 0:128],
                                     mels_sb[:, s + dk:s + dk + 512],
                                     start=(dk == 0), stop=(dk == 2))
                # relu(x + b) on DVE, cast to bf16
                nc.vector.tensor_scalar(yq1a[:, s:s + 512], qps[:],
                                        qb1a_sb[:], 0.0, ALU.add, ALU.max)
                qpsb = psq.tile([32, 512], f32, tag="qps")
                for dk in range(3):
                    nc.tensor.matmul(qpsb[:], wq1_sb[:, dk, 128:160],
                                     mels_sb[:, s + dk:s + dk + 512],
                                     start=(dk == 0), stop=(dk == 2))
                nc.vector.tensor_scalar(yq1b[:, s:s + 512], qpsb[:],
                                        qb1b_sb[:], 0.0, ALU.add, ALU.max)

            for nt in range(2):
                s = nt * 512
                qps2 = psq.tile([80, 512], f32, tag="qps")
                nc.tensor.matmul(qps2[:], wq2a_sb[:], yq1a[:, s:s + 512],
                                 start=True, stop=False)
                nc.tensor.matmul(qps2[:], wq2b_sb[:], yq1b[:, s:s + 512],
                                 start=False, stop=True)
                nc.vector.tensor_scalar(yq2[:, s:s + 512], qps2[:],
                                        qb2_sb[:], 0.0, ALU.add, ALU.max)

            for nt in range(2):
                s = nt * 512
                qps3 = psq.tile([80, 512], f32, tag="qps")
                nc.tensor.matmul(qps3[:], wq3_sb[:], yq2[:, s:s + 512],
                                 start=True, stop=True)
                nc.scalar.activation(qs[:, s:s + 512], qps3[:],
                                     AF.Identity, bias=qb3_sb[:])

            # ---- key encoder: text [512,256] -> k_aug [81,256] f32 -----------
            # k_aug rows 0..79 = k, row 80 = -0.0005 * sum_c k^2.
            y1_sb = acp.tile([128, 8, 256], bf16)
            k_sb = acp.tile([80, 256], f32)
            kkneg = acp.tile([1, 256], f32)
            ksq = acp.tile([80, 256], f32)
            kpsum = psk.tile([80, 256], f32, tag="psk")

            for co in range(8):
                kps = ps.tile([128, 256], f32, tag="big")
                first = True
                for dk in range(3):
                    for ci in range(4):
                        nc.tensor.matmul(kps[:],
                                         w1k_sb[:, dk * 4 + ci,
                                                co * 128:(co + 1) * 128],
                                         text_sb[:, ci, dk:dk + 256],
                                         start=first, stop=(dk == 2 and ci == 3))
                        first = False
                nc.vector.tensor_scalar(y1_sb[:, co, :], kps[:],
                                        kb1_sb[:, co:co + 1], 0.0,
                                        ALU.add, ALU.max)
                nc.tensor.matmul(kpsum[:], w2k_sb[:, co, :], y1_sb[:, co, :],
                                 start=(co == 0), stop=(co == 7))

            nc.scalar.activation(k_sb[:], kpsum[:], AF.Identity,
                                 bias=kb2_sb[:])
            nc.vector.tensor_tensor(ksq[:], k_sb[:], k_sb[:], ALU.mult)
            kkps = psk.tile([1, 256], f32, tag="psk")
            nc.tensor.matmul(kkps[:], ones80[:], ksq[:], start=True, stop=True)
            nc.scalar.mul(kkneg[:], kkps[:], -0.0005)

            # ---- attention + softmax per Tm-tile -----------------------------
            for i in range(8):
                r0 = i * 128
                prior_t = sm.tile([128, 256], f32)
                nc.sync.dma_start(prior_t[:], d_prior[r0:r0 + 128, :])
                # t1m = (prior + 1e-8) * mask
                t1m_t = sm.tile([128, 256], f32)
                nc.vector.scalar_tensor_tensor(t1m_t[:], prior_t[:], 1e-8,
                                               mask_sb[:], ALU.add, ALU.mult)
                # lnp = ln(prior + 1e-8)
                lnp_t = sm.tile([128, 256], f32)
                nc.scalar.activation(lnp_t[:], prior_t[:], AF.Ln, bias=eps_sb[:])

                zps = ps.tile([128, 256], f32, tag="big")
                nc.tensor.matmul(zps[:], qs[:, r0:r0 + 128], k_sb[:],
                                 start=True, stop=False)
                nc.tensor.matmul(zps[:], ones128[:], kkneg[:],
                                 start=False, stop=True)

                # e = exp(z), ssum = sum_tt e
                e_t = sm.tile([128, 256], f32)
                ssum_t = sm.tile([128, 1], f32)
                nc.scalar.activation(e_t[:], zps[:], AF.Exp, accum_out=ssum_t[:])
                # nlse = ln(1/ssum) = -logsumexp(z)
                rcp_t = sm.tile([128, 1], f32)
                nc.vector.reciprocal(rcp_t[:], ssum_t[:])
                nlse_t = sm.tile([128, 1], f32)
                nc.scalar.activation(nlse_t[:], rcp_t[:], AF.Ln)

                # logprob = (z + nlse) + lnp
                out1_t = sm.tile([128, 256], f32)
                nc.vector.scalar_tensor_tensor(out1_t[:], zps[:], nlse_t[:],
                                               lnp_t[:], ALU.add, ALU.add)
                nc.sync.dma_start(d_olp[r0:r0 + 128, :], out1_t[:])

                # h = e * t1m, denom = sum_tt h; attention = h / denom
                h_t = sm.tile([128, 256], f32)
                denom_t = sm.tile([128, 1], f32)
                nc.vector.scalar_tensor_tensor(h_t[:], e_t[:], 1.0, t1m_t[:],
                                               ALU.mult, ALU.mult,
                                               accum_out=denom_t[:])
                rec_t = sm.tile([128, 1], f32)
                nc.vector.reciprocal(rec_t[:], denom_t[:])
                out2_t = sm.tile([128, 256], f32)
                nc.scalar.mul(out2_t[:], h_t[:], rec_t[:])
                nc.sync.dma_start(d_oatt[r0:r0 + 128, :], out2_t[:])

    nc.compile()
    return nc


def _prep_inputs(text, mels, mask, attention_prior,
                 kw1, kb1, kw2, kb2, qw1, qb1, qw2, qb2, qw3, qb3):
    """Host-side shard + layout prep. Returns in_maps (one dict per core)."""
    text = np.asarray(text, np.float32)
    mels = np.asarray(mels, np.float32)
    maskf = np.asarray(mask).astype(np.float32)
    prior = np.asarray(attention_prior, np.float32)
    kw1 = np.asarray(kw1, np.float32)
    kw2 = np.asarray(kw2, np.float32)
    qw1 = np.asarray(qw1, np.float32)
    qw2 = np.asarray(qw2, np.float32)
    qw3 = np.asarray(qw3, np.float32)

    # lhsT weight layouts (shared across cores)
    t = kw1.transpose(1, 2, 0).reshape(4, 128, 3, 1024)       # [ci, p, dk, co]
    w1k_h = np.ascontiguousarray(
        t.transpose(1, 2, 0, 3).reshape(128, 12, 1024)).astype(BF)
    w2k_h = np.ascontiguousarray(
        kw2[:, :, 0].T.reshape(8, 128, 80).transpose(1, 0, 2)).astype(BF)
    wq1_h = np.ascontiguousarray(qw1.transpose(1, 2, 0)).astype(BF)
    wq2_h = np.ascontiguousarray(qw2[:, :, 0].T).astype(BF)
    wq3_h = np.ascontiguousarray(0.001 * qw3[:, :, 0].T).astype(BF)
    kb1_h = np.ascontiguousarray(
        np.asarray(kb1, np.float32).reshape(8, 128).T)
    kb2_h = np.asarray(kb2, np.float32).reshape(80, 1)
    qb1_h = np.asarray(qb1, np.float32).reshape(160, 1)
    qb2_h = np.asarray(qb2, np.float32).reshape(80, 1)
    qb3_h = (0.001 * np.asarray(qb3, np.float32)).reshape(80, 1)

    in_maps = []
    for b in range(B):
        th = np.zeros((128, 4, 258), BF)
        th[:, :, 1:257] = text[b].reshape(4, 128, 256).transpose(1, 0, 2).astype(BF)
        mh = np.zeros((80, 1026), BF)
        mh[:, 1:1025] = mels[b].astype(BF)
        mrep = np.ascontiguousarray(
            np.broadcast_to(maskf[b, 0][None, :], (128, 256)))
        in_maps.append({
            "text_pad": th,
            "mels_pad": mh,
            "w1k": w1k_h,
            "w2k": w2k_h,
            "wq1": wq1_h,
            "wq2": wq2_h,
            "wq3s": wq3_h,
            "kb1c": kb1_h,
            "kb2c": kb2_h,
            "qb1c": qb1_h,
            "qb2c": qb2_h,
            "qb3c": qb3_h,
            "maskrep": mrep,
            "prior": np.ascontiguousarray(prior[b]),
        })
    return in_maps


def run(inputs, trace=False):
    """Compile (cached), run on 8 NeuronCores, gather. Returns
    ((attention, logprob), BassKernelResults)."""
    from concourse import bass_utils

    if "nc" not in _STATE:
        _STATE["nc"] = _build()
    nc = _STATE["nc"]

    in_maps = _prep_inputs(**inputs)
    res = bass_utils.run_bass_kernel_spmd(
        nc, in_maps, core_ids=list(range(N_CORES)), trace=trace)

    att = np.stack([np.asarray(res.results[b]["out_att"]) for b in range(B)])
    lp = np.stack([np.asarray(res.results[b]["out_lp"]) for b in range(B)])
    return (att, lp), res


def kernel(**inputs):
    (att, lp), _ = run(inputs)
    return att, lp


if __name__ == "__main__":
    # smoke test with random data
    rng = np.random.default_rng(0)
    inputs = {
        "text": rng.standard_normal((B, CTXT, TT), np.float32),
        "mels": rng.standard_normal((B, CMEL, TM), np.float32),
        "mask": rng.integers(0, 2, (B, 1, TT)) > 0,
        "attention_prior": rng.random((B, TM, TT), np.float32),
        "kw1": 0.03 * rng.standard_normal((1024, 512, 3), np.float32),
        "kb1": np.zeros(1024, np.float32),
        "kw2": 0.03 * rng.standard_normal((80, 1024, 1), np.float32),
        "kb2": np.zeros(80, np.float32),
        "qw1": 0.1 * rng.standard_normal((160, 80, 3), np.float32),
        "qb1": np.zeros(160, np.float32),
        "qw2": 0.1 * rng.standard_normal((80, 160, 1), np.float32),
        "qb2": np.zeros(80, np.float32),
        "qw3": 0.1 * rng.standard_normal((80, 80, 1), np.float32),
        "qb3": np.zeros(80, np.float32),
    }
    out = kernel(**inputs)
    print("ok", out[0].shape, out[1].shape)


# revision 8
# speedup vs baseline: 1.0848x; 1.0848x over previous
"""Trainium2 Bass kernel for nn_ConvAttention.

Module: key encoder (Conv 512->1024 k3 -> ReLU -> Conv 1024->80 k1) on text,
query encoder (Conv 80->160 k3 -> ReLU -> Conv 160->80 -> ReLU -> Conv 80->80)
on mels, L2-distance attention [B,Tm,Tt], log_softmax over Tt + log prior,
masked softmax.  Returns (attention, attention_logprob), both [8,1024,256] f32.

Sharding: data-parallel over batch B=8 -> one batch item per NeuronCore;
conv weights replicated (host-prepped into lhsT layouts, bf16).

Math notes (validated numerically against the jax reference):
  - sum_c (q-k)^2 = qq + kk - 2 qk; the qq term is constant along Tt so it
    cancels exactly in log_softmax -> never computed.
  - z = 0.001*qk - 0.0005*kk: one K=80 matmul per Tm-tile (0.001 folded into
    the third query conv's weights on host) plus one K=1 rank-1 matmul that
    broadcasts -0.0005*kk across partitions, accumulated in the same PSUM.
  - z in [-0.11, 0.0] for this input distribution -> exp() needs no
    max-subtraction (identical result up to fp rounding).
  - g = exp(z) * (prior + 1e-8):
      attention_logprob = ln(g / sum_tt exp(z))   [ACT Ln with scale=1/ssum]
      attention = (g * mask) / sum_tt(g * mask)
  - ACT uses only {Exp, Ln, Copy, Relu}: Ln+Exp share one table set and
    Copy/Relu are filler in every set -> no ACT_TABLE_LOAD churn.
  - attention matmuls run as float32r (full PE rate at N>=256); their
    operands are produced with f32r output dtype as the verifier requires.
"""

import sys

sys.path.insert(0, "/opt/trn_rl_repo")

import numpy as np
import ml_dtypes

BF = ml_dtypes.bfloat16

B, CMEL, CTXT, TM, TT = 8, 80, 512, 1024, 256
N_CORES = 8

# bf16 pack free-dim offsets (per-partition element offsets)
_TEXT_O, _TEXT_N = 0, 4 * 258            # [128, 4, 258]
_MELS_O, _MELS_N = _TEXT_O + _TEXT_N, 1026   # [80, 1026]
_WQ1_O, _WQ1_N = _MELS_O + _MELS_N, 3 * 160  # [80, 3, 160]
_WQ2A_O, _WQ2A_N = _WQ1_O + _WQ1_N, 80       # [128, 80]
_WQ2B_O, _WQ2B_N = _WQ2A_O + _WQ2A_N, 80     # [32, 80]
_WQ3_O, _WQ3_N = _WQ2B_O + _WQ2B_N, 80       # [80, 80]
_W2K_O, _W2K_N = _WQ3_O + _WQ3_N, 8 * 80     # [128, 8, 80]
_PKBF_N = _W2K_O + _W2K_N

# f32 pack offsets
_KB1_O, _KB1_N = 0, 8          # [128, 8]
_KB2_O, _KB2_N = 8, 1          # [80, 1]
_QB1A_O, _QB1A_N = 9, 1        # [128, 1]
_QB1B_O, _QB1B_N = 10, 1       # [32, 1]
_QB2_O, _QB2_N = 11, 1         # [80, 1]
_QB3_O, _QB3_N = 12, 1         # [80, 1]
_MASK_O, _MASK_N = 13, 256     # [128, 256]
_PKF_N = _MASK_O + _MASK_N

_STATE = {}


def _build():
    """Build + bacc-compile the single-core program (shared by all 8 cores)."""
    import concourse.bacc as bacc
    import concourse.tile as tile
    from concourse import mybir

    f32 = mybir.dt.float32
    bf16 = mybir.dt.bfloat16
    f32r = mybir.dt.float32r
    AF = mybir.ActivationFunctionType
    ALU = mybir.AluOpType

    nc = bacc.Bacc("TRN2", target_bir_lowering=False, debug=False,
                   num_devices=N_CORES)

    d_pkbf = nc.dram_tensor("pkbf", [128, _PKBF_N], bf16, kind="ExternalInput").ap()
    d_pkf = nc.dram_tensor("pkf", [128, _PKF_N], f32, kind="ExternalInput").ap()
    # w1k: [128, chunk=4, g=12, m=256] — chunk c covers cout tiles 2c, 2c+1
    d_w1k = nc.dram_tensor("w1k", [128, 4, 12 * 256], bf16, kind="ExternalInput").ap()
    d_prior = nc.dram_tensor("prior", [128, 8, 256], f32, kind="ExternalInput").ap()
    d_oatt = nc.dram_tensor("out_att", [128, 8, 256], f32, kind="ExternalOutput").ap()
    d_olp = nc.dram_tensor("out_lp", [128, 8, 256], f32, kind="ExternalOutput").ap()

    with tile.TileContext(nc) as tc:
        with (
            tc.tile_pool(name="w", bufs=1) as wp,
            tc.tile_pool(name="act", bufs=1) as acp,
            tc.tile_pool(name="sm", bufs=3) as sm,
            tc.tile_pool(name="ps", bufs=3, space="PSUM") as ps,
            tc.tile_pool(name="psq", bufs=2, space="PSUM") as psq,
            tc.tile_pool(name="psk", bufs=2, space="PSUM") as psk,
        ):
            # ---- input loads: 7 big contiguous DMAs -------------------------
            pkbf = wp.tile([128, _PKBF_N], bf16)
            nc.sync.dma_start(pkbf[:], d_pkbf[:])
            pkf = wp.tile([128, _PKF_N], f32)
            nc.sync.dma_start(pkf[:], d_pkf[:])
            w1k_sb = wp.tile([128, 4, 12 * 256], bf16)
            for c in range(4):
                nc.sync.dma_start(w1k_sb[:, c, :], d_w1k[:, c, :])
            prior_sb = wp.tile([128, 8, 256], f32)
            nc.sync.dma_start(prior_sb[:], d_prior[:])

            # views into the packs
            text_v = pkbf[:, _TEXT_O:_TEXT_O + _TEXT_N].rearrange(
                "p (c t) -> p c t", c=4)
            mels_v = pkbf[0:80, _MELS_O:_MELS_O + _MELS_N]
            wq1_v = pkbf[0:80, _WQ1_O:_WQ1_O + _WQ1_N].rearrange(
                "p (k m) -> p k m", k=3)
            wq2a_v = pkbf[:, _WQ2A_O:_WQ2A_O + _WQ2A_N]
            wq2b_v = pkbf[0:32, _WQ2B_O:_WQ2B_O + _WQ2B_N]
            wq3_v = pkbf[0:80, _WQ3_O:_WQ3_O + _WQ3_N]
            w2k_v = pkbf[:, _W2K_O:_W2K_O + _W2K_N].rearrange(
                "p (c m) -> p c m", c=8)
            w1k_v = w1k_sb[:].rearrange("p c (g m) -> p c g m", g=12)
            kb1_v = pkf[:, _KB1_O:_KB1_O + _KB1_N]
            kb2_v = pkf[0:80, _KB2_O:_KB2_O + 1]
            qb1a_v = pkf[:, _QB1A_O:_QB1A_O + 1]
            qb1b_v = pkf[0:32, _QB1B_O:_QB1B_O + 1]
            qb2_v = pkf[0:80, _QB2_O:_QB2_O + 1]
            qb3_v = pkf[0:80, _QB3_O:_QB3_O + 1]
            mask_v = pkf[:, _MASK_O:_MASK_O + _MASK_N]

            ones_f32 = wp.tile([128, 128], f32)
            nc.vector.memset(ones_f32[:], 1.0)
            ones128 = wp.tile([1, 128], f32r)
            nc.vector.tensor_copy(ones128[:], ones_f32[0:1, :])
            ones80 = wp.tile([80, 1], f32r)
            nc.vector.tensor_copy(ones80[:], ones_f32[0:80, 0:1])

            # ---- query encoder: mels [80,1024] -> qs [80,1024] f32r ---------
            yq1a = acp.tile([128, 1024], bf16)
            yq1b = acp.tile([32, 1024], bf16)
            yq2 = acp.tile([80, 1024], bf16)
            qs = acp.tile([80, 1024], f32r)

            for nt in range(2):
                s = nt * 512
                qps = psq.tile([128, 512], f32, tag="qps")
                for dk in range(3):
                    nc.tensor.matmul(qps[:], wq1_v[:, dk, 0:128],
                                     mels_v[:, s + dk:s + dk + 512],
                                     start=(dk == 0), stop=(dk == 2))
                nc.vector.tensor_scalar(yq1a[:, s:s + 512], qps[:],
                                        qb1a_v, 0.0, ALU.add, ALU.max)
                qpsb = psq.tile([32, 512], f32, tag="qps")
                for dk in range(3):
                    nc.tensor.matmul(qpsb[:], wq1_v[:, dk, 128:160],
                                     mels_v[:, s + dk:s + dk + 512],
                                     start=(dk == 0), stop=(dk == 2))
                nc.vector.tensor_scalar(yq1b[:, s:s + 512], qpsb[:],
                                        qb1b_v, 0.0, ALU.add, ALU.max)

            for nt in range(2):
                s = nt * 512
                qps2 = psq.tile([80, 512], f32, tag="qps")
                nc.tensor.matmul(qps2[:], wq2a_v, yq1a[:, s:s + 512],
                                 start=True, stop=False)
                nc.tensor.matmul(qps2[:], wq2b_v, yq1b[:, s:s + 512],
                                 start=False, stop=True)
                nc.vector.tensor_scalar(yq2[:, s:s + 512], qps2[:],
                                        qb2_v, 0.0, ALU.add, ALU.max)

            for nt in range(2):
                s = nt * 512
                qps3 = psq.tile([80, 512], f32, tag="qps")
                nc.tensor.matmul(qps3[:], wq3_v, yq2[:, s:s + 512],
                                 start=True, stop=True)
                # qs = qps3 + qb3 (f32r rounded on write)
                nc.vector.tensor_scalar_add(qs[:, s:s + 512], qps3[:], qb3_v)

            # ---- key encoder: text [512,256] -> k [80,256] f32r -------------
            y1_sb = acp.tile([128, 8, 256], bf16)
            k_sb = acp.tile([80, 256], f32r)
            kkneg = acp.tile([1, 256], f32r)
            ksq = acp.tile([80, 256], f32r)
            olp_sb = acp.tile([128, 8, 256], f32)
            oatt_sb = acp.tile([128, 8, 256], f32)
            kpsum = psk.tile([80, 256], f32, tag="psk")

            for co in range(8):
                kps = ps.tile([128, 256], f32, tag="big")
                first = True
                for dk in range(3):
                    for ci in range(4):
                        nc.tensor.matmul(
                            kps[:],
                            w1k_v[:, co // 2, dk * 4 + ci,
                                  (co % 2) * 128:(co % 2) * 128 + 128],
                            text_v[:, ci, dk:dk + 256],
                            start=first, stop=(dk == 2 and ci == 3))
                        first = False
                # relu(x + b) on ACT (Relu is filler in every table set)
                nc.scalar.activation(y1_sb[:, co, :], kps[:], AF.Relu,
                                     bias=kb1_v[:, co:co + 1])
                nc.tensor.matmul(kpsum[:], w2k_v[:, co, :], y1_sb[:, co, :],
                                 start=(co == 0), stop=(co == 7))

            nc.vector.tensor_scalar_add(k_sb[:], kpsum[:], kb2_v)
            nc.vector.tensor_tensor(ksq[:], k_sb[:], k_sb[:], ALU.mult)
            kkps = psk.tile([1, 256], f32, tag="psk")
            nc.tensor.matmul(kkps[:], ones80[:], ksq[:], start=True, stop=True)
            nc.vector.tensor_scalar_mul(kkneg[:], kkps[:], -0.0005)

            # ---- attention + softmax per Tm-tile ----------------------------
            for i in range(8):
                zps = ps.tile([128, 256], f32, tag="big")
                nc.tensor.matmul(zps[:], qs[:, i * 128:(i + 1) * 128], k_sb[:],
                                 start=True, stop=False)
                nc.tensor.matmul(zps[:], ones128[:], kkneg[:],
                                 start=False, stop=True)

                # e = exp(z), ssum = sum_tt e
                e_t = sm.tile([128, 256], f32)
                ssum_t = sm.tile([128, 1], f32)
                nc.scalar.activation(e_t[:], zps[:], AF.Exp, accum_out=ssum_t[:])
                rcp_t = sm.tile([128, 1], f32)
                nc.vector.reciprocal(rcp_t[:], ssum_t[:])

                # g = (prior + 1e-8) * e
                g_t = sm.tile([128, 256], f32)
                nc.vector.scalar_tensor_tensor(g_t[:], prior_sb[:, i, :], 1e-8,
                                               e_t[:], ALU.add, ALU.mult)
                # logprob = ln(g / ssum)
                nc.scalar.activation(olp_sb[:, i, :], g_t[:], AF.Ln,
                                     scale=rcp_t[:])

                # h = g * mask, denom = sum_tt h; attention = h / denom
                h_t = sm.tile([128, 256], f32)
                denom_t = sm.tile([128, 1], f32)
                nc.vector.scalar_tensor_tensor(h_t[:], g_t[:], 1.0, mask_v,
                                               ALU.mult, ALU.mult,
                                               accum_out=denom_t[:])
                rec_t = sm.tile([128, 1], f32)
                nc.vector.reciprocal(rec_t[:], denom_t[:])
                nc.scalar.mul(oatt_sb[:, i, :], h_t[:], rec_t[:])

            # output DMAs in halves
            nc.sync.dma_start(d_olp[:, 0:4, :], olp_sb[:, 0:4, :])
            nc.sync.dma_start(d_oatt[:, 0:4, :], oatt_sb[:, 0:4, :])
            nc.sync.dma_start(d_olp[:, 4:8, :], olp_sb[:, 4:8, :])
            nc.sync.dma_start(d_oatt[:, 4:8, :], oatt_sb[:, 4:8, :])

    nc.compile()
    return nc


def _prep_shared(kw1, kb1, kw2, kb2, qw1, qb1, qw2, qb2, qw3, qb3):
    """Weight/bias layout prep shared across cores."""
    kw1 = np.asarray(kw1, np.float32)
    kw2 = np.asarray(kw2, np.float32)
    qw1 = np.asarray(qw1, np.float32)
    qw2 = np.asarray(qw2, np.float32)
    qw3 = np.asarray(qw3, np.float32)

    # w1k host layout [p, chunk, g, m]: chunk c + local m -> cout c*256+m,
    # g = dk*4+ci, p = cin within tile ci.
    t = kw1.transpose(1, 2, 0).reshape(4, 128, 3, 1024)   # [ci, p, dk, co]
    w1k = t.transpose(1, 2, 0, 3).reshape(128, 12, 4, 256)  # [p, g, chunk, m]
    w1k_h = np.ascontiguousarray(
        w1k.transpose(0, 2, 1, 3).reshape(128, 4, 12 * 256)).astype(BF)

    pk_bf_shared = {
        "wq1": (slice(0, 80), _WQ1_O, qw1.transpose(1, 2, 0).reshape(80, -1)),
        "wq2a": (slice(0, 128), _WQ2A_O, qw2[:, :, 0].T[0:128]),
        "wq2b": (slice(0, 32), _WQ2B_O, qw2[:, :, 0].T[128:160]),
        "wq3": (slice(0, 80), _WQ3_O, 0.001 * qw3[:, :, 0].T),
        "w2k": (slice(0, 128), _W2K_O,
                kw2[:, :, 0].T.reshape(8, 128, 80).transpose(1, 0, 2)
                .reshape(128, -1)),
    }
    pkf_shared = {
        "kb1": (slice(0, 128), _KB1_O,
                np.asarray(kb1, np.float32).reshape(8, 128).T),
        "kb2": (slice(0, 80), _KB2_O,
                np.asarray(kb2, np.float32).reshape(80, 1)),
        "qb1a": (slice(0, 128), _QB1A_O,
                 np.asarray(qb1, np.float32)[0:128].reshape(128, 1)),
        "qb1b": (slice(0, 32), _QB1B_O,
                 np.asarray(qb1, np.float32)[128:160].reshape(32, 1)),
        "qb2": (slice(0, 80), _QB2_O,
                np.asarray(qb2, np.float32).reshape(80, 1)),
        "qb3": (slice(0, 80), _QB3_O,
                (0.001 * np.asarray(qb3, np.float32)).reshape(80, 1)),
    }
    return w1k_h, pk_bf_shared, pkf_shared


def _prep_inputs(text, mels, mask, attention_prior, **weights):
    """Host-side shard + layout prep. Returns in_maps (one dict per core)."""
    text = np.asarray(text, np.float32)
    mels = np.asarray(mels, np.float32)
    maskf = np.asarray(mask).astype(np.float32)
    prior = np.asarray(attention_prior, np.float32)

    w1k_h, pk_bf_shared, pkf_shared = _prep_shared(**weights)

    pkf0 = np.zeros((128, _PKF_N), np.float32)
    for rows, off, arr in pkf_shared.values():
        pkf0[rows, off:off + arr.shape[1]] = arr

    in_maps = []
    for b in range(B):
        pkbf = np.zeros((128, _PKBF_N), BF)
        tp = pkbf[:, _TEXT_O:_TEXT_O + _TEXT_N].reshape(128, 4, 258)
        tp[:, :, 1:257] = text[b].reshape(4, 128, 256).transpose(1, 0, 2).astype(BF)
        mp = pkbf[0:80, _MELS_O:_MELS_O + _MELS_N]
        mp[:, 1:1025] = mels[b].astype(BF)
        for rows, off, arr in pk_bf_shared.values():
            pkbf[rows, off:off + arr.shape[1]] = arr.astype(BF)

        pkf = pkf0.copy()
        pkf[:, _MASK_O:_MASK_O + _MASK_N] = maskf[b, 0][None, :]

        # prior p-major: [p, co, t] = prior[co*128+p, t]
        prior_p = np.ascontiguousarray(
            prior[b].reshape(8, 128, 256).transpose(1, 0, 2))

        in_maps.append({
            "pkbf": pkbf,
            "pkf": pkf,
            "w1k": w1k_h,
            "prior": prior_p,
        })
    return in_maps


def run(inputs, trace=False):
    """Compile (cached), run on 8 NeuronCores, gather. Returns
    ((attention, logprob), BassKernelResults)."""
    from concourse import bass_utils

    if "nc" not in _STATE:
        _STATE["nc"] = _build()
    nc = _STATE["nc"]

    in_maps = _prep_inputs(**inputs)
    res = bass_utils.run_bass_kernel_spmd(
        nc, in_maps, core_ids=list(range(N_CORES)), trace=trace)

    # outputs are p-major [128, 8, 256] -> [1024, 256]
    def unp(a):
        return np.asarray(a).transpose(1, 0, 2).reshape(1024, 256)

    att = np.stack([unp(res.results[b]["out_att"]) for b in range(B)])
    lp = np.stack([unp(res.results[b]["out_lp"]) for b in range(B)])
    return (att, lp), res


def kernel(**inputs):
    (att, lp), _ = run(inputs)
    return att, lp


if __name__ == "__main__":
    rng = np.random.default_rng(0)
    inputs = {
        "text": rng.standard_normal((B, CTXT, TT)).astype(np.float32),
        "mels": rng.standard_normal((B, CMEL, TM)).astype(np.float32),
        "mask": rng.integers(0, 2, (B, 1, TT)) > 0,
        "attention_prior": rng.random((B, TM, TT)).astype(np.float32),
        "kw1": (0.03 * rng.standard_normal((1024, 512, 3))).astype(np.float32),
        "kb1": np.zeros(1024, np.float32),
        "kw2": (0.03 * rng.standard_normal((80, 1024, 1))).astype(np.float32),
        "kb2": np.zeros(80, np.float32),
        "qw1": (0.1 * rng.standard_normal((160, 80, 3))).astype(np.float32),
        "qb1": np.zeros(160, np.float32),
        "qw2": (0.1 * rng.standard_normal((80, 160, 1))).astype(np.float32),
        "qb2": np.zeros(80, np.float32),
        "qw3": (0.1 * rng.standard_normal((80, 80, 1))).astype(np.float32),
        "qb3": np.zeros(80, np.float32),
    }
    out = kernel(**inputs)
    print("ok", out[0].shape, out[1].shape)


# revision 9
# speedup vs baseline: 1.2240x; 1.1283x over previous
"""Trainium2 Bass kernel for nn_ConvAttention.

Module: key encoder (Conv 512->1024 k3 -> ReLU -> Conv 1024->80 k1) on text,
query encoder (Conv 80->160 k3 -> ReLU -> Conv 160->80 -> ReLU -> Conv 80->80)
on mels, L2-distance attention [B,Tm,Tt], log_softmax over Tt + log prior,
masked softmax.  Returns (attention, attention_logprob), both [8,1024,256] f32.

Sharding: data-parallel over batch B=8 -> one batch item per NeuronCore;
conv weights replicated (host-prepped into lhsT layouts, bf16).

Math notes (validated numerically against the jax reference):
  - sum_c (q-k)^2 = qq + kk - 2 qk; the qq term is constant along Tt so it
    cancels exactly in log_softmax -> never computed.
  - z = 0.001*qk - 0.0005*kk: one K=80 matmul per Tm-tile (0.001 folded into
    the third query conv's weights on host) plus one K=1 rank-1 matmul that
    broadcasts -0.0005*kk across partitions, accumulated in the same PSUM.
  - z in [-0.11, 0.0] for this input distribution -> exp() needs no
    max-subtraction (identical result up to fp rounding).
  - g = exp(z) * (prior + 1e-8):
      attention_logprob = ln(g / sum_tt exp(z))   [ACT Ln with scale=1/ssum]
      attention = (g * mask) / sum_tt(g * mask)
  - ACT uses only {Exp, Ln, Copy, Relu}: Ln+Exp share one table set and
    Copy/Relu are filler in every set -> no ACT_TABLE_LOAD churn.
  - attention matmuls run as float32r (full PE rate at N>=256); their
    operands are produced with f32r output dtype as the verifier requires.
"""

import sys

sys.path.insert(0, "/opt/trn_rl_repo")

import numpy as np
import ml_dtypes

BF = ml_dtypes.bfloat16

B, CMEL, CTXT, TM, TT = 8, 80, 512, 1024, 256
N_CORES = 8

# bf16 pack free-dim offsets (per-partition element offsets)
_TEXT_O, _TEXT_N = 0, 4 * 258            # [128, 4, 258]
_MELS_O, _MELS_N = _TEXT_O + _TEXT_N, 1026   # [80, 1026]
_WQ1_O, _WQ1_N = _MELS_O + _MELS_N, 3 * 160  # [80, 3, 160]
_WQ2A_O, _WQ2A_N = _WQ1_O + _WQ1_N, 80       # [128, 80]
_WQ2B_O, _WQ2B_N = _WQ2A_O + _WQ2A_N, 80     # [32, 80]
_WQ3_O, _WQ3_N = _WQ2B_O + _WQ2B_N, 80       # [80, 80]
_W2K_O, _W2K_N = _WQ3_O + _WQ3_N, 8 * 80     # [128, 8, 80]
_PKBF_N = _W2K_O + _W2K_N

# f32 pack offsets
_KB1_O, _KB1_N = 0, 8          # [128, 8]
_KB2_O, _KB2_N = 8, 1          # [80, 1]
_QB1A_O, _QB1A_N = 9, 1        # [128, 1]
_QB1B_O, _QB1B_N = 10, 1       # [32, 1]
_QB2_O, _QB2_N = 11, 1         # [80, 1]
_QB3_O, _QB3_N = 12, 1         # [80, 1]
_MASK_O, _MASK_N = 13, 256     # [128, 256]
_PKF_N = _MASK_O + _MASK_N

_STATE = {}


def _build():
    """Build + bacc-compile the single-core program (shared by all 8 cores)."""
    import concourse.bacc as bacc
    import concourse.tile as tile
    from concourse import mybir

    f32 = mybir.dt.float32
    bf16 = mybir.dt.bfloat16
    f32r = mybir.dt.float32r
    AF = mybir.ActivationFunctionType
    ALU = mybir.AluOpType

    nc = bacc.Bacc("TRN2", target_bir_lowering=False, debug=False,
                   num_devices=N_CORES)

    d_pkbf = nc.dram_tensor("pkbf", [128, _PKBF_N], bf16, kind="ExternalInput").ap()
    d_pkf = nc.dram_tensor("pkf", [128, _PKF_N], f32, kind="ExternalInput").ap()
    # w1k: [128, chunk=4, g=12, m=256] — chunk c covers cout tiles 2c, 2c+1
    d_w1k = nc.dram_tensor("w1k", [128, 4, 12 * 256], bf16, kind="ExternalInput").ap()
    d_prior = nc.dram_tensor("prior", [128, 8, 256], f32, kind="ExternalInput").ap()
    d_oatt = nc.dram_tensor("out_att", [128, 8, 256], f32, kind="ExternalOutput").ap()
    d_olp = nc.dram_tensor("out_lp", [128, 8, 256], f32, kind="ExternalOutput").ap()

    with tile.TileContext(nc) as tc:
        with (
            tc.tile_pool(name="w", bufs=1) as wp,
            tc.tile_pool(name="act", bufs=1) as acp,
            tc.tile_pool(name="sm", bufs=3) as sm,
            tc.tile_pool(name="ps", bufs=3, space="PSUM") as ps,
            tc.tile_pool(name="psq", bufs=2, space="PSUM") as psq,
            tc.tile_pool(name="psk", bufs=2, space="PSUM") as psk,
        ):
            # ---- input loads: 7 big contiguous DMAs -------------------------
            pkbf = wp.tile([128, _PKBF_N], bf16)
            nc.sync.dma_start(pkbf[:], d_pkbf[:])
            pkf = wp.tile([128, _PKF_N], f32)
            nc.sync.dma_start(pkf[:], d_pkf[:])
            from concourse.tile_rust import add_dep_helper
            w1k_sb = wp.tile([128, 4, 12 * 256], bf16)
            prev = None
            for c in range(4):
                dch = nc.sync.dma_start(w1k_sb[:, c, :], d_w1k[:, c, :])
                if prev is not None:
                    add_dep_helper(dch.ins, prev.ins, sync=True,
                                   reason="serialize w1k chunks for early conv1")
                prev = dch
            prior_sb = wp.tile([128, 8, 256], f32)
            dpr = nc.sync.dma_start(prior_sb[:], d_prior[:])
            add_dep_helper(dpr.ins, prev.ins, sync=True,
                           reason="prior after w1k (needed later)")

            # views into the packs
            text_v = pkbf[:, _TEXT_O:_TEXT_O + _TEXT_N].rearrange(
                "p (c t) -> p c t", c=4)
            mels_v = pkbf[0:80, _MELS_O:_MELS_O + _MELS_N]
            wq1_v = pkbf[0:80, _WQ1_O:_WQ1_O + _WQ1_N].rearrange(
                "p (k m) -> p k m", k=3)
            wq2a_v = pkbf[:, _WQ2A_O:_WQ2A_O + _WQ2A_N]
            wq2b_v = pkbf[0:32, _WQ2B_O:_WQ2B_O + _WQ2B_N]
            wq3_v = pkbf[0:80, _WQ3_O:_WQ3_O + _WQ3_N]
            w2k_v = pkbf[:, _W2K_O:_W2K_O + _W2K_N].rearrange(
                "p (c m) -> p c m", c=8)
            w1k_v = w1k_sb[:].rearrange("p c (g m) -> p c g m", g=12)
            kb1_v = pkf[:, _KB1_O:_KB1_O + _KB1_N]
            kb2_v = pkf[0:80, _KB2_O:_KB2_O + 1]
            qb1a_v = pkf[:, _QB1A_O:_QB1A_O + 1]
            qb1b_v = pkf[0:32, _QB1B_O:_QB1B_O + 1]
            qb2_v = pkf[0:80, _QB2_O:_QB2_O + 1]
            qb3_v = pkf[0:80, _QB3_O:_QB3_O + 1]
            mask_v = pkf[:, _MASK_O:_MASK_O + _MASK_N]

            ones_f32 = wp.tile([128, 128], f32)
            nc.vector.memset(ones_f32[:], 1.0)
            ones128 = wp.tile([1, 128], f32r)
            nc.vector.tensor_copy(ones128[:], ones_f32[0:1, :])
            ones80 = wp.tile([80, 1], f32r)
            nc.vector.tensor_copy(ones80[:], ones_f32[0:80, 0:1])

            # ---- query encoder: mels [80,1024] -> qs [80,1024] f32r ---------
            yq1a = acp.tile([128, 1024], bf16)
            yq1b = acp.tile([32, 1024], bf16)
            yq2 = acp.tile([80, 1024], bf16)
            qs = acp.tile([80, 1024], f32r)

            for nt in range(2):
                s = nt * 512
                qps = psq.tile([128, 512], f32, tag="qps")
                for dk in range(3):
                    nc.tensor.matmul(qps[:], wq1_v[:, dk, 0:128],
                                     mels_v[:, s + dk:s + dk + 512],
                                     start=(dk == 0), stop=(dk == 2))
                nc.vector.tensor_scalar(yq1a[:, s:s + 512], qps[:],
                                        qb1a_v, 0.0, ALU.add, ALU.max)
                qpsb = psq.tile([32, 512], f32, tag="qps")
                for dk in range(3):
                    nc.tensor.matmul(qpsb[:], wq1_v[:, dk, 128:160],
                                     mels_v[:, s + dk:s + dk + 512],
                                     start=(dk == 0), stop=(dk == 2))
                nc.vector.tensor_scalar(yq1b[:, s:s + 512], qpsb[:],
                                        qb1b_v, 0.0, ALU.add, ALU.max)

            for nt in range(2):
                s = nt * 512
                qps2 = psq.tile([80, 512], f32, tag="qps")
                nc.tensor.matmul(qps2[:], wq2a_v, yq1a[:, s:s + 512],
                                 start=True, stop=False)
                nc.tensor.matmul(qps2[:], wq2b_v, yq1b[:, s:s + 512],
                                 start=False, stop=True)
                nc.vector.tensor_scalar(yq2[:, s:s + 512], qps2[:],
                                        qb2_v, 0.0, ALU.add, ALU.max)

            for nt in range(2):
                s = nt * 512
                qps3 = psq.tile([80, 512], f32, tag="qps")
                nc.tensor.matmul(qps3[:], wq3_v, yq2[:, s:s + 512],
                                 start=True, stop=True)
                # qs = qps3 + qb3 (f32r rounded on write)
                nc.vector.tensor_scalar_add(qs[:, s:s + 512], qps3[:], qb3_v)

            # ---- key encoder: text [512,256] -> k [80,256] f32r -------------
            y1_sb = acp.tile([128, 8, 256], bf16)
            k_sb = acp.tile([80, 256], f32r)
            kkneg = acp.tile([1, 256], f32r)
            ksq = acp.tile([80, 256], f32r)
            olp_sb = acp.tile([128, 8, 256], f32)
            oatt_sb = acp.tile([128, 8, 256], f32)
            kpsum = psk.tile([80, 256], f32, tag="psk")

            for co in range(8):
                kps = ps.tile([128, 256], f32, tag="big")
                first = True
                for dk in range(3):
                    for ci in range(4):
                        nc.tensor.matmul(
                            kps[:],
                            w1k_v[:, co // 2, dk * 4 + ci,
                                  (co % 2) * 128:(co % 2) * 128 + 128],
                            text_v[:, ci, dk:dk + 256],
                            start=first, stop=(dk == 2 and ci == 3))
                        first = False
                # relu(x + b) on ACT (Relu is filler in every table set)
                nc.scalar.activation(y1_sb[:, co, :], kps[:], AF.Relu,
                                     bias=kb1_v[:, co:co + 1])
                nc.tensor.matmul(kpsum[:], w2k_v[:, co, :], y1_sb[:, co, :],
                                 start=(co == 0), stop=(co == 7))

            nc.vector.tensor_scalar_add(k_sb[:], kpsum[:], kb2_v)
            nc.vector.tensor_tensor(ksq[:], k_sb[:], k_sb[:], ALU.mult)
            kkps = psk.tile([1, 256], f32, tag="psk")
            nc.tensor.matmul(kkps[:], ones80[:], ksq[:], start=True, stop=True)
            nc.vector.tensor_scalar_mul(kkneg[:], kkps[:], -0.0005)

            # ---- attention + softmax, phase-batched so ACT runs
            # 8x EXP, then 8x LN, then 8x COPY (no table churn) ------------
            e_all = acp.tile([128, 8, 256], f32)
            g_all = acp.tile([128, 8, 256], f32)
            h_all = acp.tile([128, 8, 256], f32)
            ssum_all = acp.tile([128, 8], f32)
            rcp_all = acp.tile([128, 8], f32)
            den_all = acp.tile([128, 8], f32)
            rec_all = acp.tile([128, 8], f32)

            for i in range(8):
                zps = ps.tile([128, 256], f32, tag="big")
                nc.tensor.matmul(zps[:], qs[:, i * 128:(i + 1) * 128], k_sb[:],
                                 start=True, stop=False)
                nc.tensor.matmul(zps[:], ones128[:], kkneg[:],
                                 start=False, stop=True)
                nc.scalar.activation(e_all[:, i, :], zps[:], AF.Exp,
                                     accum_out=ssum_all[:, i:i + 1])
            for i in range(8):
                nc.vector.reciprocal(rcp_all[:, i:i + 1], ssum_all[:, i:i + 1])
                nc.vector.scalar_tensor_tensor(g_all[:, i, :], prior_sb[:, i, :],
                                               1e-8, e_all[:, i, :],
                                               ALU.add, ALU.mult)
            for i in range(8):
                nc.scalar.activation(olp_sb[:, i, :], g_all[:, i, :], AF.Ln,
                                     scale=rcp_all[:, i:i + 1])
            for i in range(8):
                nc.vector.scalar_tensor_tensor(h_all[:, i, :], g_all[:, i, :],
                                               1.0, mask_v, ALU.mult, ALU.mult,
                                               accum_out=den_all[:, i:i + 1])
                nc.vector.reciprocal(rec_all[:, i:i + 1], den_all[:, i:i + 1])
            for i in range(8):
                nc.scalar.mul(oatt_sb[:, i, :], h_all[:, i, :],
                              rec_all[:, i:i + 1])

            # output DMAs in halves
            nc.sync.dma_start(d_olp[:, 0:4, :], olp_sb[:, 0:4, :])
            nc.sync.dma_start(d_oatt[:, 0:4, :], oatt_sb[:, 0:4, :])
            nc.sync.dma_start(d_olp[:, 4:8, :], olp_sb[:, 4:8, :])
            nc.sync.dma_start(d_oatt[:, 4:8, :], oatt_sb[:, 4:8, :])

    nc.compile()
    return nc


def _prep_shared(kw1, kb1, kw2, kb2, qw1, qb1, qw2, qb2, qw3, qb3):
    """Weight/bias layout prep shared across cores."""
    kw1 = np.asarray(kw1, np.float32)
    kw2 = np.asarray(kw2, np.float32)
    qw1 = np.asarray(qw1, np.float32)
    qw2 = np.asarray(qw2, np.float32)
    qw3 = np.asarray(qw3, np.float32)

    # w1k host layout [p, chunk, g, m]: chunk c + local m -> cout c*256+m,
    # g = dk*4+ci, p = cin within tile ci.
    t = kw1.transpose(1, 2, 0).reshape(4, 128, 3, 1024)   # [ci, p, dk, co]
    w1k = t.transpose(1, 2, 0, 3).reshape(128, 12, 4, 256)  # [p, g, chunk, m]
    w1k_h = np.ascontiguousarray(
        w1k.transpose(0, 2, 1, 3).reshape(128, 4, 12 * 256)).astype(BF)

    pk_bf_shared = {
        "wq1": (slice(0, 80), _WQ1_O, qw1.transpose(1, 2, 0).reshape(80, -1)),
        "wq2a": (slice(0, 128), _WQ2A_O, qw2[:, :, 0].T[0:128]),
        "wq2b": (slice(0, 32), _WQ2B_O, qw2[:, :, 0].T[128:160]),
        "wq3": (slice(0, 80), _WQ3_O, 0.001 * qw3[:, :, 0].T),
        "w2k": (slice(0, 128), _W2K_O,
                kw2[:, :, 0].T.reshape(8, 128, 80).transpose(1, 0, 2)
                .reshape(128, -1)),
    }
    pkf_shared = {
        "kb1": (slice(0, 128), _KB1_O,
                np.asarray(kb1, np.float32).reshape(8, 128).T),
        "kb2": (slice(0, 80), _KB2_O,
                np.asarray(kb2, np.float32).reshape(80, 1)),
        "qb1a": (slice(0, 128), _QB1A_O,
                 np.asarray(qb1, np.float32)[0:128].reshape(128, 1)),
        "qb1b": (slice(0, 32), _QB1B_O,
                 np.asarray(qb1, np.float32)[128:160].reshape(32, 1)),
        "qb2": (slice(0, 80), _QB2_O,
                np.asarray(qb2, np.float32).reshape(80, 1)),
        "qb3": (slice(0, 80), _QB3_O,
                (0.001 * np.asarray(qb3, np.float32)).reshape(80, 1)),
    }
    return w1k_h, pk_bf_shared, pkf_shared


def _prep_inputs(text, mels, mask, attention_prior, **weights):
    """Host-side shard + layout prep. Returns in_maps (one dict per core)."""
    text = np.asarray(text, np.float32)
    mels = np.asarray(mels, np.float32)
    maskf = np.asarray(mask).astype(np.float32)
    prior = np.asarray(attention_prior, np.float32)

    w1k_h, pk_bf_shared, pkf_shared = _prep_shared(**weights)

    pkf0 = np.zeros((128, _PKF_N), np.float32)
    for rows, off, arr in pkf_shared.values():
        pkf0[rows, off:off + arr.shape[1]] = arr

    in_maps = []
    for b in range(B):
        pkbf = np.zeros((128, _PKBF_N), BF)
        tp = pkbf[:, _TEXT_O:_TEXT_O + _TEXT_N].reshape(128, 4, 258)
        tp[:, :, 1:257] = text[b].reshape(4, 128, 256).transpose(1, 0, 2).astype(BF)
        mp = pkbf[0:80, _MELS_O:_MELS_O + _MELS_N]
        mp[:, 1:1025] = mels[b].astype(BF)
        for rows, off, arr in pk_bf_shared.values():
            pkbf[rows, off:off + arr.shape[1]] = arr.astype(BF)

        pkf = pkf0.copy()
        pkf[:, _MASK_O:_MASK_O + _MASK_N] = maskf[b, 0][None, :]

        # prior p-major: [p, co, t] = prior[co*128+p, t]
        prior_p = np.ascontiguousarray(
            prior[b].reshape(8, 128, 256).transpose(1, 0, 2))

        in_maps.append({
            "pkbf": pkbf,
            "pkf": pkf,
            "w1k": w1k_h,
            "prior": prior_p,
        })
    return in_maps


def run(inputs, trace=False):
    """Compile (cached), run on 8 NeuronCores, gather. Returns
    ((attention, logprob), BassKernelResults)."""
    from concourse import bass_utils

    if "nc" not in _STATE:
        _STATE["nc"] = _build()
    nc = _STATE["nc"]

    in_maps = _prep_inputs(**inputs)
    res = bass_utils.run_bass_kernel_spmd(
        nc, in_maps, core_ids=list(range(N_CORES)), trace=trace)

    # outputs are p-major [128, 8, 256] -> [1024, 256]
    def unp(a):
        return np.asarray(a).transpose(1, 0, 2).reshape(1024, 256)

    att = np.stack([unp(res.results[b]["out_att"]) for b in range(B)])
    lp = np.stack([unp(res.results[b]["out_lp"]) for b in range(B)])
    return (att, lp), res


def kernel(**inputs):
    (att, lp), _ = run(inputs)
    return att, lp


if __name__ == "__main__":
    rng = np.random.default_rng(0)
    inputs = {
        "text": rng.standard_normal((B, CTXT, TT)).astype(np.float32),
        "mels": rng.standard_normal((B, CMEL, TM)).astype(np.float32),
        "mask": rng.integers(0, 2, (B, 1, TT)) > 0,
        "attention_prior": rng.random((B, TM, TT)).astype(np.float32),
        "kw1": (0.03 * rng.standard_normal((1024, 512, 3))).astype(np.float32),
        "kb1": np.zeros(1024, np.float32),
        "kw2": (0.03 * rng.standard_normal((80, 1024, 1))).astype(np.float32),
        "kb2": np.zeros(80, np.float32),
        "qw1": (0.1 * rng.standard_normal((160, 80, 3))).astype(np.float32),
        "qb1": np.zeros(160, np.float32),
        "qw2": (0.1 * rng.standard_normal((80, 160, 1))).astype(np.float32),
        "qb2": np.zeros(80, np.float32),
        "qw3": (0.1 * rng.standard_normal((80, 80, 1))).astype(np.float32),
        "qb3": np.zeros(80, np.float32),
    }
    out = kernel(**inputs)
    print("ok", out[0].shape, out[1].shape)
